# revision 28
# baseline (speedup 1.0000x reference)
"""Causal multi-head attention on 8 TRN2 NeuronCores.

Problem: x[4,2048,1024], w_attn[1024,3072], w_proj[1024,1024],
16 heads x 64 dim, causal softmax(QK^T/8)V then output projection.

Sharding: 4-way batch x 2-way head-half. Core c handles batch c//2 and
heads (c%2)*8 .. (c%2)*8+8. Each core computes a partial y^T (its head
half's contribution to the output projection); the host sums the two
partials per batch and transposes.

v3 design (probe-calibrated):
 - ACT does ONLY softmax exps (it is the 2nd-busiest engine, ~170us);
   every other PSUM evacuation lives on DVE; gpsimd is never used
   (software engine, can't read PSUM).
 - All QKV-projection work ("stage A") is sliced into single-matmul
   quanta and PUMPED into the attention pipeline one chunk ahead
   (chunk n+1's projections run inside chunk n's S/PV stream), so the
   PE never sits idle while ACT chews exps and there is no serial
   projection prologue except for chunk 0.
 - S^T[k,q] via K=64 pairs at partition bases 0/64 (run concurrently
   on the PE's row tiles; probe: 275ns/pair = same as one K=128 op).
 - PV stationary is [V_h | ones*64]: PSUM rows 64:128 of the PV output
   all receive the softmax denominator -> free partition broadcast.
   Normalize = reciprocal_approx_fast (5x faster than reciprocal,
   ~18 bits) + one DVE multiply straight out of PSUM.
 - Diagonal causal chunks: S/exp/PV restricted to [off:512]; the
   128-wide diagonal strip masked with a triangular multiply on DVE.
 - PSUM budget (8 banks): 3 x "s" (S + exp), 2 x "a" (projection
   accumulators), 3 x "o" (PV accumulators + output projection).
"""

import math
import numpy as np
from contextlib import ExitStack

import concourse.bass as bass
import concourse.tile as tile
from concourse import bacc, mybir
from concourse.bass_utils import run_bass_kernel_spmd

f32 = mybir.dt.float32
bf16 = mybir.dt.bfloat16
EXP = mybir.ActivationFunctionType.Exp
COPY = mybir.ActivationFunctionType.Copy

B, T, C = 4, 2048, 1024
N_HEAD, HD = 16, 64
HPC = 8            # heads per core
FS = HPC * HD      # 512: per-core feature slice for each of q/k/v
NPAIR = HPC // 2   # 4 head pairs
SCALE = 1.0 / 8.0  # 1/sqrt(64)
N_CORES = 8


def build_nc(tpc=T, loop_n=1, dyn_loop=0, stages='ABC', apump=True):
    """Build the single-core Bass program (SPMD: same program all cores).

    apump=False is a diagnostic mode: all projection work runs as a serial
    prologue instead of being pumped into the attention pipeline."""
    nck = C // 128          # 8 c_in tiles
    nkt = tpc // 128        # key tiles
    nqc = tpc // 512        # query chunks (512 wide)
    nmt = C // 128          # 8 output-channel tiles

    nc = bacc.Bacc("TRN2", target_bir_lowering=False)
    xt = nc.dram_tensor("xt", [C, tpc], bf16, kind="ExternalInput")
    wq = nc.dram_tensor("wq", [C, FS], bf16, kind="ExternalInput")
    wk = nc.dram_tensor("wk", [C, FS], bf16, kind="ExternalInput")
    wv = nc.dram_tensor("wv", [C, FS], bf16, kind="ExternalInput")
    wp = nc.dram_tensor("wp", [FS, C], bf16, kind="ExternalInput")
    mk = nc.dram_tensor("mk", [128, 128], bf16, kind="ExternalInput")
    on = nc.dram_tensor("on", [128, nkt * HPC * HD], bf16, kind="ExternalInput")
    yt = nc.dram_tensor("yt", [C, tpc], f32, kind="ExternalOutput")

    with tile.TileContext(nc) as tc, ExitStack() as _dl:
     if dyn_loop:
        _dl.enter_context(tc.For_i(0, dyn_loop, 1))
     for _rep in range(loop_n):
      with ExitStack() as stk:
        persist = stk.enter_context(tc.tile_pool(name="persist", bufs=1))
        # K^T feature-major for all pairs; vhat with ones columns
        kT = persist.tile([128, NPAIR, tpc], bf16, tag="kT", name="kT")
        vhat = persist.tile([128, nkt, HPC, 128], bf16, tag="vh", name="vhat")
        mkt = persist.tile([128, 128], bf16, tag="mk", name="mkt")
        wqk = persist.tile([128, nck, 2, FS], bf16, tag="wqk", name="wqk")
        wvt = persist.tile([128, nck, FS], bf16, tag="wv", name="wvt")
        wpt = persist.tile([128, NPAIR, nmt, 128], bf16, tag="wp", name="wpt")
        xts = [persist.tile([128, tpc], bf16, tag=f"x{i}", name=f"x{i}")
               for i in range(nck)]

        # input DMAs: x chunk-0 slices first so chunk-0 projections start
        # ASAP, then weights, then the rest of x
        for i in range(nck):
            nc.sync.dma_start(out=xts[i][:, 0:512], in_=xt[i * 128:(i + 1) * 128, 0:512])
        nc.sync.dma_start(out=wqk[:, :, 0, :], in_=wq.rearrange("(a p) f -> p a f", p=128))
        nc.sync.dma_start(out=wqk[:, :, 1, :], in_=wk.rearrange("(a p) f -> p a f", p=128))
        # ones in columns 0:64 -> PV output rows 0:64 carry the softmax
        # denominator at partition base 0 (required by recip_approx ucode)
        nc.sync.dma_start(
            out=vhat.rearrange("p a h w -> p (a h) w")[:, :, 0:HD],
            in_=on.rearrange("p (g w) -> p g w", w=HD))
        nc.sync.dma_start(out=wvt, in_=wv.rearrange("(a p) f -> p a f", p=128))
        nc.sync.dma_start(out=mkt, in_=mk[:, :])
        nc.sync.dma_start(
            out=wpt, in_=wp.rearrange("(a p) (m w) -> p a m w", p=128, w=128))
        for n in range(1, nqc):
            for i in range(nck):
                nc.sync.dma_start(out=xts[i][:, n * 512:(n + 1) * 512],
                                  in_=xt[i * 128:(i + 1) * 128, n * 512:(n + 1) * 512])

        with tc.tile_pool(name="qTcp", bufs=2) as qTcp, \
             tc.tile_pool(name="oTcp", bufs=2) as oTcp, \
             tc.tile_pool(name="pp", bufs=40) as pp, \
             tc.tile_pool(name="rp", bufs=4) as rp, \
             tc.tile_pool(name="ev", bufs=3) as ev, \
             tc.tile_pool(name="psS", bufs=4, space="PSUM") as psS, \
             tc.tile_pool(name="psA", bufs=2, space="PSUM") as psA, \
             tc.tile_pool(name="psO", bufs=2, space="PSUM") as psO:

            qTcs = {}   # chunk -> qT tile [128, NPAIR, 512]
            oTcs = {}   # chunk -> oT tile [128, NPAIR, 512]

            # ---- projection-work feeder (quantized stage A) ----
            aq = {"items": []}

            def push_group(stat_fn, mov_fn, evac_fn):
                st = {}
                for k in range(nck):
                    def mm(k=k, st=st):
                        if k == 0:
                            st["ps"] = psA.tile([128, 512], f32, tag="a",
                                                name="aps")
                        nc.tensor.matmul(st["ps"][:, :], stat_fn(k), mov_fn(k),
                                         start=(k == 0), stop=(k == nck - 1))
                    aq["items"].append(mm)

                def evac(st=st):
                    evac_fn(st["ps"])
                aq["items"].append(evac)

            def push_chunk(n):
                qTc = qTcp.tile([128, NPAIR, 512], bf16, tag="qTc",
                                name=f"qTc{n}")
                qTcs[n] = qTc
                nsl = slice(n * 512, (n + 1) * 512)
                for d in (1, 0):          # K groups first, then Q
                    for m in range(NPAIR):
                        def evac_qk(ps, d=d, m=m, qTc=qTc, nsl=nsl):
                            out = (kT[:, m, nsl] if d == 1 else qTc[:, m, :])
                            nc.vector.tensor_copy(out, ps[:, :])
                        push_group(
                            lambda k, d=d, m=m: wqk[:, k, d, m * 128:(m + 1) * 128],
                            lambda k, nsl=nsl: xts[k][:, nsl],
                            evac_qk)
                for vt in range(4):       # V token tiles of this chunk
                    t = 4 * n + vt
                    def evac_v(ps, t=t):
                        nc.vector.tensor_copy(
                            vhat[:, t, :, HD:128],
                            ps[:, :].rearrange("p (h d) -> p h d", h=HPC))
                    push_group(
                        lambda k, t=t: xts[k][:, t * 128:(t + 1) * 128],
                        lambda k: wvt[:, k, :],
                        evac_v)

            def pump(q):
                for _ in range(q):
                    if aq["items"]:
                        aq["items"].pop(0)()

            # ---- attention unit machinery ----
            def new_state(p, qc):
                kts = list(range(min(nkt, 4 * (qc + 1))))
                return {"p": p, "qc": qc, "kts": kts, "i": 0, "j": 0,
                        "masked": 0, "ptiles": [], "po": None}

            def emit_s_step(st):
                p, qc, kts, i = st["p"], st["qc"], st["kts"], st["i"]
                if i >= len(kts):
                    return False
                kt = kts[i]
                ksl = slice(kt * 128, (kt + 1) * 128)
                diag = (kt // 4 == qc)
                off = 128 * (kt % 4) if diag else 0
                prs = []
                for par in range(2):   # head parity: partitions 0/64
                    row = slice(64 * par, 64 * par + 64)
                    ps = psS.tile([128, 512], f32, tag="s", name="s")
                    nc.tensor.matmul(
                        ps[:, off:512], kT[row, p, ksl],
                        qTcs[qc][row, p, off:512], start=True, stop=True)
                    pr = pp.tile([128, 512], bf16, tag="P", name="P")
                    nc.scalar.activation(pr[:, off:512], ps[:, off:512],
                                         EXP, scale=SCALE)
                    if diag:  # mask the 128-wide diagonal strip
                        nc.vector.tensor_mul(
                            pr[:, off:off + 128],
                            pr[:, off:off + 128], mkt[:, :])
                    prs.append(pr)
                st["ptiles"].append((prs, off))
                st["i"] += 1
                return True

            def emit_pv_step(st):
                p, kts, j = st["p"], st["kts"], st["j"]
                if j >= len(kts):
                    return False
                if st["po"] is None:
                    st["po"] = [psO.tile([128, 512], f32, tag="o", name="po")
                                for _ in range(2)]
                kt = kts[j]
                prs, off = st["ptiles"][j]
                for par in range(2):
                    nc.tensor.matmul(
                        st["po"][par][:, off:512],
                        vhat[:, kt, 2 * p + par, :],
                        prs[par][:, off:512],
                        start=(j == 0), stop=(j == len(kts) - 1),
                        skip_group_check=True)
                st["j"] += 1
                return True

            def emit_norm(st):
                p, qc = st["p"], st["qc"]
                for par in range(2):
                    po = st["po"][par]
                    # recip_approx is a custom-ucode DVE op: it only works at
                    # partition base 0 -> den lives in po rows 0:64, O in
                    # rows 64:128 (vhat is [ones | V])
                    bcr = rp.tile([128, 512], f32, tag="bcr", name="bcr")
                    nc.vector.reciprocal_approx_fast(bcr[0:HD, :],
                                                     po[0:HD, :])
                    nc.vector.tensor_mul(
                        oTcs[qc][64 * par:64 * par + 64, p, :],
                        po[HD:128, :], bcr[0:HD, :])

            def emit_c_chunk(n):
                oTc = oTcs[n]
                for m in range(nmt):
                    ps = psO.tile([128, 512], f32, tag="o", name="cps")
                    for j in range(NPAIR):
                        nc.tensor.matmul(
                            ps[:, :], wpt[:, j, m, :], oTc[:, j, :],
                            start=(j == 0), stop=(j == NPAIR - 1))
                    sb = ev.tile([128, 512], f32, tag="sb", name="sb")
                    nc.vector.tensor_copy(sb[:, :], ps[:, :])
                    nc.sync.dma_start(
                        out=yt[m * 128:(m + 1) * 128, n * 512:(n + 1) * 512],
                        in_=sb)

            def retire(st):
                while emit_pv_step(st):
                    pass
                emit_norm(st)
                if 'C' in stages and st["p"] == NPAIR - 1:
                    emit_c_chunk(st["qc"])

            # ---- main schedule ----
            units_on = 'B' in stages
            push_chunk(0)
            pump(10 ** 9)              # chunk-0 projections are the prologue
            if not apump:
                for n in range(1, nqc):
                    push_chunk(n)
                pump(10 ** 9)
            prev = None
            for n in range(nqc):
                if not units_on:
                    if apump and n + 1 < nqc:
                        push_chunk(n + 1)
                        pump(10 ** 9)
                    continue
                oTcs[n] = oTcp.tile([128, NPAIR, 512], bf16, tag="oTc",
                                    name=f"oTc{n}")
                if apump and n + 1 < nqc:
                    push_chunk(n + 1)
                steps_left = 4 * NPAIR * (n + 1)
                for p in range(NPAIR):
                    cur = new_state(p, n)
                    while True:
                        if prev is not None:
                            emit_pv_step(prev)
                        if not emit_s_step(cur):
                            break
                        rate = math.ceil(len(aq["items"]) / max(steps_left, 1))
                        pump(rate)
                        steps_left -= 1
                    if prev is not None:
                        retire(prev)
                    prev = cur
            if prev is not None:
                retire(prev)
            pump(10 ** 9)
    nc.compile()
    return nc


def _make_masks():
    import ml_dtypes
    k = np.arange(128)[:, None]
    q = np.arange(128)[None, :]
    return (q >= k).astype(ml_dtypes.bfloat16)


_NC_CACHE = {}


def _get_nc(tpc=T):
    if tpc not in _NC_CACHE:
        _NC_CACHE[tpc] = build_nc(tpc)
    return _NC_CACHE[tpc]


def make_in_maps(x, w_attn, w_proj):
    import ml_dtypes
    bf = ml_dtypes.bfloat16
    masks = _make_masks()
    nkt = T // 128
    ones = np.ones((128, nkt * HPC * HD), dtype=bf)
    in_maps = []
    for core in range(N_CORES):
        b, hh = core // 2, core % 2
        s = slice(hh * FS, (hh + 1) * FS)
        in_maps.append({
            "xt": np.ascontiguousarray(np.asarray(x[b]).T).astype(bf),
            "wq": np.ascontiguousarray(w_attn[:, s]).astype(bf),
            "wk": np.ascontiguousarray(w_attn[:, C:][:, s]).astype(bf),
            "wv": np.ascontiguousarray(w_attn[:, 2 * C:][:, s]).astype(bf),
            "wp": np.ascontiguousarray(w_proj[hh * FS:(hh + 1) * FS, :]).astype(bf),
            "mk": masks,
            "on": ones,
        })
    return in_maps


def kernel(x, w_attn, w_proj):
    nc = _get_nc(T)
    in_maps = make_in_maps(x, w_attn, w_proj)
    res = run_bass_kernel_spmd(nc, in_maps, list(range(N_CORES)))
    y = np.empty((B, T, C), np.float32)
    for b in range(B):
        yt = res.results[2 * b]["yt"] + res.results[2 * b + 1]["yt"]
        y[b] = yt.T
    return y


# revision 30
# speedup vs baseline: 1.0552x; 1.0552x over previous
"""Causal multi-head attention on 8 TRN2 NeuronCores.

Problem: x[4,2048,1024], w_attn[1024,3072], w_proj[1024,1024],
16 heads x 64 dim, causal softmax(QK^T/8)V then output projection.

Sharding: 4-way batch x 2-way head-half. Core c handles batch c//2 and
heads (c%2)*8 .. (c%2)*8+8. Each core computes a partial y^T (its head
half's contribution to the output projection); the host sums the two
partials per batch and transposes.

v3 design (probe-calibrated):
 - ACT does ONLY softmax exps (it is the 2nd-busiest engine, ~170us);
   every other PSUM evacuation lives on DVE; gpsimd is never used
   (software engine, can't read PSUM).
 - All QKV-projection work ("stage A") is sliced into single-matmul
   quanta and PUMPED into the attention pipeline one chunk ahead
   (chunk n+1's projections run inside chunk n's S/PV stream), so the
   PE never sits idle while ACT chews exps and there is no serial
   projection prologue except for chunk 0.
 - S^T[k,q] via K=64 pairs at partition bases 0/64 (run concurrently
   on the PE's row tiles; probe: 275ns/pair = same as one K=128 op).
 - PV stationary is [V_h | ones*64]: PSUM rows 64:128 of the PV output
   all receive the softmax denominator -> free partition broadcast.
   Normalize = reciprocal_approx_fast (5x faster than reciprocal,
   ~18 bits) + one DVE multiply straight out of PSUM.
 - Diagonal causal chunks: S/exp/PV restricted to [off:512]; the
   128-wide diagonal strip masked with a triangular multiply on DVE.
 - PSUM budget (8 banks): 3 x "s" (S + exp), 2 x "a" (projection
   accumulators), 3 x "o" (PV accumulators + output projection).
"""

import math
import numpy as np
from contextlib import ExitStack

import concourse.bass as bass
import concourse.tile as tile
from concourse import bacc, mybir
from concourse.bass_utils import run_bass_kernel_spmd

f32 = mybir.dt.float32
bf16 = mybir.dt.bfloat16
EXP = mybir.ActivationFunctionType.Exp
COPY = mybir.ActivationFunctionType.Copy

B, T, C = 4, 2048, 1024
N_HEAD, HD = 16, 64
HPC = 8            # heads per core
FS = HPC * HD      # 512: per-core feature slice for each of q/k/v
NPAIR = HPC // 2   # 4 head pairs
SCALE = 1.0 / 8.0  # 1/sqrt(64)
N_CORES = 8


def build_nc(tpc=T, loop_n=1, dyn_loop=0, stages='ABC', apump=True):
    """Build the single-core Bass program (SPMD: same program all cores).

    apump=False is a diagnostic mode: all projection work runs as a serial
    prologue instead of being pumped into the attention pipeline."""
    nck = C // 128          # 8 c_in tiles
    nkt = tpc // 128        # key tiles
    nqc = tpc // 512        # query chunks (512 wide)
    nmt = C // 128          # 8 output-channel tiles

    nc = bacc.Bacc("TRN2", target_bir_lowering=False)
    xt = nc.dram_tensor("xt", [C, tpc], bf16, kind="ExternalInput")
    wq = nc.dram_tensor("wq", [C, FS], bf16, kind="ExternalInput")
    wk = nc.dram_tensor("wk", [C, FS], bf16, kind="ExternalInput")
    wv = nc.dram_tensor("wv", [C, FS], bf16, kind="ExternalInput")
    wp = nc.dram_tensor("wp", [FS, C], bf16, kind="ExternalInput")
    mk = nc.dram_tensor("mk", [128, 128], bf16, kind="ExternalInput")
    on = nc.dram_tensor("on", [128, nkt * HPC * HD], bf16, kind="ExternalInput")
    yt = nc.dram_tensor("yt", [C, tpc], f32, kind="ExternalOutput")

    with tile.TileContext(nc) as tc, ExitStack() as _dl:
     if dyn_loop:
        _dl.enter_context(tc.For_i(0, dyn_loop, 1))
     for _rep in range(loop_n):
      with ExitStack() as stk:
        persist = stk.enter_context(tc.tile_pool(name="persist", bufs=1))
        # K^T feature-major for all pairs; vhat with ones columns
        kT = persist.tile([128, NPAIR, tpc], bf16, tag="kT", name="kT")
        vhat = persist.tile([128, nkt, HPC, 128], bf16, tag="vh", name="vhat")
        mkt = persist.tile([128, 128], bf16, tag="mk", name="mkt")
        wqk = persist.tile([128, nck, 2, FS], bf16, tag="wqk", name="wqk")
        wvt = persist.tile([128, nck, FS], bf16, tag="wv", name="wvt")
        wpt = persist.tile([128, NPAIR, nmt, 128], bf16, tag="wp", name="wpt")
        xts = [persist.tile([128, tpc], bf16, tag=f"x{i}", name=f"x{i}")
               for i in range(nck)]

        # input DMAs: x chunk-0 slices first so chunk-0 projections start
        # ASAP, then weights, then the rest of x
        for i in range(nck):
            nc.sync.dma_start(out=xts[i][:, 0:512], in_=xt[i * 128:(i + 1) * 128, 0:512])
        nc.sync.dma_start(out=wqk[:, :, 0, :], in_=wq.rearrange("(a p) f -> p a f", p=128))
        nc.sync.dma_start(out=wqk[:, :, 1, :], in_=wk.rearrange("(a p) f -> p a f", p=128))
        # ones in columns 0:64 -> PV output rows 0:64 carry the softmax
        # denominator at partition base 0 (required by recip_approx ucode)
        nc.sync.dma_start(
            out=vhat.rearrange("p a h w -> p (a h) w")[:, :, 0:HD],
            in_=on.rearrange("p (g w) -> p g w", w=HD))
        nc.sync.dma_start(out=wvt, in_=wv.rearrange("(a p) f -> p a f", p=128))
        nc.sync.dma_start(out=mkt, in_=mk[:, :])
        nc.sync.dma_start(
            out=wpt, in_=wp.rearrange("(a p) (m w) -> p a m w", p=128, w=128))
        for n in range(1, nqc):
            for i in range(nck):
                nc.sync.dma_start(out=xts[i][:, n * 512:(n + 1) * 512],
                                  in_=xt[i * 128:(i + 1) * 128, n * 512:(n + 1) * 512])

        with tc.tile_pool(name="qTcp", bufs=2) as qTcp, \
             tc.tile_pool(name="oTcp", bufs=2) as oTcp, \
             tc.tile_pool(name="pp", bufs=46) as pp, \
             tc.tile_pool(name="rp", bufs=6) as rp, \
             tc.tile_pool(name="ev", bufs=4) as ev, \
             tc.tile_pool(name="psS", bufs=3, space="PSUM") as psS, \
             tc.tile_pool(name="psA", bufs=2, space="PSUM") as psA, \
             tc.tile_pool(name="psO", bufs=3, space="PSUM") as psO:

            qTcs = {}   # chunk -> qT tile [128, NPAIR, 512]
            oTcs = {}   # chunk -> oT tile [128, NPAIR, 512]

            # ---- projection-work feeder (quantized stage A) ----
            aq = {"items": []}

            def push_group(stat_fn, mov_fn, evac_fn):
                st = {}
                for k in range(nck):
                    def mm(k=k, st=st):
                        if k == 0:
                            st["ps"] = psA.tile([128, 512], f32, tag="a",
                                                name="aps")
                        nc.tensor.matmul(st["ps"][:, :], stat_fn(k), mov_fn(k),
                                         start=(k == 0), stop=(k == nck - 1))
                    aq["items"].append(mm)

                def evac(st=st):
                    evac_fn(st["ps"])
                aq["items"].append(evac)

            def push_chunk(n):
                qTc = qTcp.tile([128, NPAIR, 512], bf16, tag="qTc",
                                name=f"qTc{n}")
                qTcs[n] = qTc
                nsl = slice(n * 512, (n + 1) * 512)
                for d in (1, 0):          # K groups first, then Q
                    for m in range(NPAIR):
                        def evac_qk(ps, d=d, m=m, qTc=qTc, nsl=nsl):
                            out = (kT[:, m, nsl] if d == 1 else qTc[:, m, :])
                            nc.vector.tensor_copy(out, ps[:, :])
                        push_group(
                            lambda k, d=d, m=m: wqk[:, k, d, m * 128:(m + 1) * 128],
                            lambda k, nsl=nsl: xts[k][:, nsl],
                            evac_qk)
                for vt in range(4):       # V token tiles of this chunk
                    t = 4 * n + vt
                    def evac_v(ps, t=t):
                        nc.vector.tensor_copy(
                            vhat[:, t, :, HD:128],
                            ps[:, :].rearrange("p (h d) -> p h d", h=HPC))
                    push_group(
                        lambda k, t=t: xts[k][:, t * 128:(t + 1) * 128],
                        lambda k: wvt[:, k, :],
                        evac_v)

            def pump(q):
                for _ in range(q):
                    if aq["items"]:
                        aq["items"].pop(0)()

            # ---- attention unit machinery ----
            def new_state(p, qc):
                kts = list(range(min(nkt, 4 * (qc + 1))))
                return {"p": p, "qc": qc, "kts": kts, "i": 0, "j": 0,
                        "masked": 0, "ptiles": [], "po": None}

            def emit_s_step(st):
                p, qc, kts, i = st["p"], st["qc"], st["kts"], st["i"]
                if i >= len(kts):
                    return False
                kt = kts[i]
                ksl = slice(kt * 128, (kt + 1) * 128)
                diag = (kt // 4 == qc)
                off = 128 * (kt % 4) if diag else 0
                prs = []
                for par in range(2):   # head parity: partitions 0/64
                    row = slice(64 * par, 64 * par + 64)
                    ps = psS.tile([128, 512], f32, tag="s", name="s")
                    nc.tensor.matmul(
                        ps[:, off:512], kT[row, p, ksl],
                        qTcs[qc][row, p, off:512], start=True, stop=True)
                    pr = pp.tile([128, 512], bf16, tag="P", name="P")
                    nc.scalar.activation(pr[:, off:512], ps[:, off:512],
                                         EXP, scale=SCALE)
                    if diag:  # mask the 128-wide diagonal strip
                        nc.vector.tensor_mul(
                            pr[:, off:off + 128],
                            pr[:, off:off + 128], mkt[:, :])
                    prs.append(pr)
                st["ptiles"].append((prs, off))
                st["i"] += 1
                return True

            def emit_pv_step(st):
                p, kts, j = st["p"], st["kts"], st["j"]
                if j >= len(kts):
                    return False
                if st["po"] is None:
                    st["po"] = [psO.tile([128, 512], f32, tag="o", name="po")
                                for _ in range(2)]
                kt = kts[j]
                prs, off = st["ptiles"][j]
                for par in range(2):
                    nc.tensor.matmul(
                        st["po"][par][:, off:512],
                        vhat[:, kt, 2 * p + par, :],
                        prs[par][:, off:512],
                        start=(j == 0), stop=(j == len(kts) - 1),
                        skip_group_check=True)
                st["j"] += 1
                return True

            def emit_norm(st):
                p, qc = st["p"], st["qc"]
                for par in range(2):
                    po = st["po"][par]
                    # recip_approx is a custom-ucode DVE op: it only works at
                    # partition base 0 -> den lives in po rows 0:64, O in
                    # rows 64:128 (vhat is [ones | V])
                    bcr = rp.tile([128, 512], f32, tag="bcr", name="bcr")
                    nc.vector.reciprocal_approx_fast(bcr[0:HD, :],
                                                     po[0:HD, :])
                    nc.vector.tensor_mul(
                        oTcs[qc][64 * par:64 * par + 64, p, :],
                        po[HD:128, :], bcr[0:HD, :])

            def emit_c_chunk(n):
                oTc = oTcs[n]
                for m in range(nmt):
                    ps = psO.tile([128, 512], f32, tag="o", name="cps")
                    for j in range(NPAIR):
                        nc.tensor.matmul(
                            ps[:, :], wpt[:, j, m, :], oTc[:, j, :],
                            start=(j == 0), stop=(j == NPAIR - 1))
                    sb = ev.tile([128, 512], f32, tag="sb", name="sb")
                    nc.vector.tensor_copy(sb[:, :], ps[:, :])
                    nc.sync.dma_start(
                        out=yt[m * 128:(m + 1) * 128, n * 512:(n + 1) * 512],
                        in_=sb)

            def retire(st):
                while emit_pv_step(st):
                    pass
                emit_norm(st)
                if 'C' in stages and st["p"] == NPAIR - 1:
                    emit_c_chunk(st["qc"])

            # ---- main schedule ----
            units_on = 'B' in stages
            push_chunk(0)
            pump(10 ** 9)              # chunk-0 projections are the prologue
            if not apump:
                for n in range(1, nqc):
                    push_chunk(n)
                pump(10 ** 9)
            prev = None
            for n in range(nqc):
                if not units_on:
                    if apump and n + 1 < nqc:
                        push_chunk(n + 1)
                        pump(10 ** 9)
                    continue
                oTcs[n] = oTcp.tile([128, NPAIR, 512], bf16, tag="oTc",
                                    name=f"oTc{n}")
                if apump and n + 1 < nqc:
                    push_chunk(n + 1)
                steps_left = 4 * NPAIR * (n + 1)
                for p in range(NPAIR):
                    cur = new_state(p, n)
                    while True:
                        if prev is not None:
                            emit_pv_step(prev)
                        if not emit_s_step(cur):
                            break
                        rate = math.ceil(len(aq["items"]) / max(steps_left, 1))
                        pump(rate)
                        steps_left -= 1
                    if prev is not None:
                        retire(prev)
                    prev = cur
            if prev is not None:
                retire(prev)
            pump(10 ** 9)
    nc.compile()
    return nc


def _make_masks():
    import ml_dtypes
    k = np.arange(128)[:, None]
    q = np.arange(128)[None, :]
    return (q >= k).astype(ml_dtypes.bfloat16)


_NC_CACHE = {}


def _get_nc(tpc=T):
    if tpc not in _NC_CACHE:
        _NC_CACHE[tpc] = build_nc(tpc)
    return _NC_CACHE[tpc]


def make_in_maps(x, w_attn, w_proj):
    import ml_dtypes
    bf = ml_dtypes.bfloat16
    masks = _make_masks()
    nkt = T // 128
    ones = np.ones((128, nkt * HPC * HD), dtype=bf)
    in_maps = []
    for core in range(N_CORES):
        b, hh = core // 2, core % 2
        s = slice(hh * FS, (hh + 1) * FS)
        in_maps.append({
            "xt": np.ascontiguousarray(np.asarray(x[b]).T).astype(bf),
            "wq": np.ascontiguousarray(w_attn[:, s]).astype(bf),
            "wk": np.ascontiguousarray(w_attn[:, C:][:, s]).astype(bf),
            "wv": np.ascontiguousarray(w_attn[:, 2 * C:][:, s]).astype(bf),
            "wp": np.ascontiguousarray(w_proj[hh * FS:(hh + 1) * FS, :]).astype(bf),
            "mk": masks,
            "on": ones,
        })
    return in_maps


def kernel(x, w_attn, w_proj):
    nc = _get_nc(T)
    in_maps = make_in_maps(x, w_attn, w_proj)
    res = run_bass_kernel_spmd(nc, in_maps, list(range(N_CORES)))
    y = np.empty((B, T, C), np.float32)
    for b in range(B):
        yt = res.results[2 * b]["yt"] + res.results[2 * b + 1]["yt"]
        y[b] = yt.T
    return y


# revision 32
# speedup vs baseline: 1.0619x; 1.0064x over previous
"""Causal multi-head attention on 8 TRN2 NeuronCores.

Problem: x[4,2048,1024], w_attn[1024,3072], w_proj[1024,1024],
16 heads x 64 dim, causal softmax(QK^T/8)V then output projection.

Sharding: 4-way batch x 2-way head-half. Core c handles batch c//2 and
heads (c%2)*8 .. (c%2)*8+8. Each core computes a partial y^T (its head
half's contribution to the output projection); the host sums the two
partials per batch and transposes.

v3 design (probe-calibrated):
 - ACT does ONLY softmax exps (it is the 2nd-busiest engine, ~170us);
   every other PSUM evacuation lives on DVE; gpsimd is never used
   (software engine, can't read PSUM).
 - All QKV-projection work ("stage A") is sliced into single-matmul
   quanta and PUMPED into the attention pipeline one chunk ahead
   (chunk n+1's projections run inside chunk n's S/PV stream), so the
   PE never sits idle while ACT chews exps and there is no serial
   projection prologue except for chunk 0.
 - S^T[k,q] via K=64 pairs at partition bases 0/64 (run concurrently
   on the PE's row tiles; probe: 275ns/pair = same as one K=128 op).
 - PV stationary is [V_h | ones*64]: PSUM rows 64:128 of the PV output
   all receive the softmax denominator -> free partition broadcast.
   Normalize = reciprocal_approx_fast (5x faster than reciprocal,
   ~18 bits) + one DVE multiply straight out of PSUM.
 - Diagonal causal chunks: S/exp/PV restricted to [off:512]; the
   128-wide diagonal strip masked with a triangular multiply on DVE.
 - PSUM budget (8 banks): 3 x "s" (S + exp), 2 x "a" (projection
   accumulators), 3 x "o" (PV accumulators + output projection).
"""

import math
import numpy as np
from contextlib import ExitStack

import concourse.bass as bass
import concourse.tile as tile
from concourse import bacc, mybir
from concourse.bass_utils import run_bass_kernel_spmd

f32 = mybir.dt.float32
bf16 = mybir.dt.bfloat16
EXP = mybir.ActivationFunctionType.Exp
COPY = mybir.ActivationFunctionType.Copy

B, T, C = 4, 2048, 1024
N_HEAD, HD = 16, 64
HPC = 8            # heads per core
FS = HPC * HD      # 512: per-core feature slice for each of q/k/v
NPAIR = HPC // 2   # 4 head pairs
SCALE = 1.0 / 8.0  # 1/sqrt(64)
N_CORES = 8


def build_nc(tpc=T, loop_n=1, dyn_loop=0, stages='ABC', apump=True):
    """Build the single-core Bass program (SPMD: same program all cores).

    apump=False is a diagnostic mode: all projection work runs as a serial
    prologue instead of being pumped into the attention pipeline."""
    nck = C // 128          # 8 c_in tiles
    nkt = tpc // 128        # key tiles
    nqc = tpc // 512        # query chunks (512 wide)
    nmt = C // 128          # 8 output-channel tiles

    nc = bacc.Bacc("TRN2", target_bir_lowering=False)
    xt = nc.dram_tensor("xt", [C, tpc], bf16, kind="ExternalInput")
    wq = nc.dram_tensor("wq", [C, FS], bf16, kind="ExternalInput")
    wk = nc.dram_tensor("wk", [C, FS], bf16, kind="ExternalInput")
    wv = nc.dram_tensor("wv", [C, FS], bf16, kind="ExternalInput")
    wp = nc.dram_tensor("wp", [FS, C], bf16, kind="ExternalInput")
    mk = nc.dram_tensor("mk", [128, 128], bf16, kind="ExternalInput")
    on = nc.dram_tensor("on", [128, nkt * HPC * HD], bf16, kind="ExternalInput")
    yt = nc.dram_tensor("yt", [C, tpc], f32, kind="ExternalOutput")

    with tile.TileContext(nc) as tc, ExitStack() as _dl:
     if dyn_loop:
        _dl.enter_context(tc.For_i(0, dyn_loop, 1))
     for _rep in range(loop_n):
      with ExitStack() as stk:
        persist = stk.enter_context(tc.tile_pool(name="persist", bufs=1))
        # K^T feature-major for all pairs; vhat with ones columns
        kT = persist.tile([128, NPAIR, tpc], bf16, tag="kT", name="kT")
        vhat = persist.tile([128, nkt, HPC, 128], bf16, tag="vh", name="vhat")
        mkt = persist.tile([128, 128], bf16, tag="mk", name="mkt")
        wqk = persist.tile([128, nck, 2, FS], bf16, tag="wqk", name="wqk")
        wvt = persist.tile([128, nck, FS], bf16, tag="wv", name="wvt")
        wpt = persist.tile([128, NPAIR, nmt, 128], bf16, tag="wp", name="wpt")
        xts = [persist.tile([128, tpc], bf16, tag=f"x{i}", name=f"x{i}")
               for i in range(nck)]

        # input DMAs: x chunk-0 slices first so chunk-0 projections start
        # ASAP, then weights, then the rest of x
        # DMA order = first-use order: prologue QKV projections need x
        # chunk 0 + wqk + wvt first; pumped chunk-1 projections need the
        # rest of x next; the ones (first PV ~25us in) and wpt (first
        # output projection ~60us in) can land last
        for i in range(nck):
            nc.sync.dma_start(out=xts[i][:, 0:512], in_=xt[i * 128:(i + 1) * 128, 0:512])
        nc.sync.dma_start(out=wqk[:, :, 0, :], in_=wq.rearrange("(a p) f -> p a f", p=128))
        nc.sync.dma_start(out=wqk[:, :, 1, :], in_=wk.rearrange("(a p) f -> p a f", p=128))
        nc.sync.dma_start(out=wvt, in_=wv.rearrange("(a p) f -> p a f", p=128))
        nc.sync.dma_start(out=mkt, in_=mk[:, :])
        for n in range(1, nqc):
            for i in range(nck):
                nc.sync.dma_start(out=xts[i][:, n * 512:(n + 1) * 512],
                                  in_=xt[i * 128:(i + 1) * 128, n * 512:(n + 1) * 512])
        # ones in columns 0:64 -> PV output rows 0:64 carry the softmax
        # denominator at partition base 0 (required by recip_approx ucode)
        nc.sync.dma_start(
            out=vhat.rearrange("p a h w -> p (a h) w")[:, :, 0:HD],
            in_=on.rearrange("p (g w) -> p g w", w=HD))
        nc.sync.dma_start(
            out=wpt, in_=wp.rearrange("(a p) (m w) -> p a m w", p=128, w=128))

        with tc.tile_pool(name="qTcp", bufs=2) as qTcp, \
             tc.tile_pool(name="oTcp", bufs=2) as oTcp, \
             tc.tile_pool(name="pp", bufs=40) as pp, \
             tc.tile_pool(name="rp", bufs=4) as rp, \
             tc.tile_pool(name="ev", bufs=3) as ev, \
             tc.tile_pool(name="psS", bufs=3, space="PSUM") as psS, \
             tc.tile_pool(name="psA", bufs=2, space="PSUM") as psA, \
             tc.tile_pool(name="psO", bufs=3, space="PSUM") as psO:

            qTcs = {}   # chunk -> qT tile [128, NPAIR, 512]
            oTcs = {}   # chunk -> oT tile [128, NPAIR, 512]

            # ---- projection-work feeder (quantized stage A) ----
            aq = {"items": []}

            def push_group(stat_fn, mov_fn, evac_fn):
                st = {}
                for k in range(nck):
                    def mm(k=k, st=st):
                        if k == 0:
                            st["ps"] = psA.tile([128, 512], f32, tag="a",
                                                name="aps")
                        nc.tensor.matmul(st["ps"][:, :], stat_fn(k), mov_fn(k),
                                         start=(k == 0), stop=(k == nck - 1))
                    aq["items"].append(mm)

                def evac(st=st):
                    evac_fn(st["ps"])
                aq["items"].append(evac)

            def push_chunk(n):
                qTc = qTcp.tile([128, NPAIR, 512], bf16, tag="qTc",
                                name=f"qTc{n}")
                qTcs[n] = qTc
                nsl = slice(n * 512, (n + 1) * 512)
                for d in (1, 0):          # K groups first, then Q
                    for m in range(NPAIR):
                        def evac_qk(ps, d=d, m=m, qTc=qTc, nsl=nsl):
                            out = (kT[:, m, nsl] if d == 1 else qTc[:, m, :])
                            nc.vector.tensor_copy(out, ps[:, :])
                        push_group(
                            lambda k, d=d, m=m: wqk[:, k, d, m * 128:(m + 1) * 128],
                            lambda k, nsl=nsl: xts[k][:, nsl],
                            evac_qk)
                for vt in range(4):       # V token tiles of this chunk
                    t = 4 * n + vt
                    def evac_v(ps, t=t):
                        nc.vector.tensor_copy(
                            vhat[:, t, :, HD:128],
                            ps[:, :].rearrange("p (h d) -> p h d", h=HPC))
                    push_group(
                        lambda k, t=t: xts[k][:, t * 128:(t + 1) * 128],
                        lambda k: wvt[:, k, :],
                        evac_v)

            def pump(q):
                for _ in range(q):
                    if aq["items"]:
                        aq["items"].pop(0)()

            # ---- attention unit machinery ----
            def new_state(p, qc):
                kts = list(range(min(nkt, 4 * (qc + 1))))
                return {"p": p, "qc": qc, "kts": kts, "i": 0, "j": 0,
                        "masked": 0, "ptiles": [], "po": None}

            def emit_s_step(st):
                p, qc, kts, i = st["p"], st["qc"], st["kts"], st["i"]
                if i >= len(kts):
                    return False
                kt = kts[i]
                ksl = slice(kt * 128, (kt + 1) * 128)
                diag = (kt // 4 == qc)
                off = 128 * (kt % 4) if diag else 0
                prs = []
                for par in range(2):   # head parity: partitions 0/64
                    row = slice(64 * par, 64 * par + 64)
                    ps = psS.tile([128, 512], f32, tag="s", name="s")
                    nc.tensor.matmul(
                        ps[:, off:512], kT[row, p, ksl],
                        qTcs[qc][row, p, off:512], start=True, stop=True)
                    pr = pp.tile([128, 512], bf16, tag="P", name="P")
                    nc.scalar.activation(pr[:, off:512], ps[:, off:512],
                                         EXP, scale=SCALE)
                    if diag:  # mask the 128-wide diagonal strip
                        nc.vector.tensor_mul(
                            pr[:, off:off + 128],
                            pr[:, off:off + 128], mkt[:, :])
                    prs.append(pr)
                st["ptiles"].append((prs, off))
                st["i"] += 1
                return True

            def emit_pv_step(st):
                p, kts, j = st["p"], st["kts"], st["j"]
                if j >= len(kts):
                    return False
                if st["po"] is None:
                    st["po"] = [psO.tile([128, 512], f32, tag="o", name="po")
                                for _ in range(2)]
                kt = kts[j]
                prs, off = st["ptiles"][j]
                for par in range(2):
                    nc.tensor.matmul(
                        st["po"][par][:, off:512],
                        vhat[:, kt, 2 * p + par, :],
                        prs[par][:, off:512],
                        start=(j == 0), stop=(j == len(kts) - 1),
                        skip_group_check=True)
                st["j"] += 1
                return True

            def emit_norm(st):
                p, qc = st["p"], st["qc"]
                for par in range(2):
                    po = st["po"][par]
                    # recip_approx is a custom-ucode DVE op: it only works at
                    # partition base 0 -> den lives in po rows 0:64, O in
                    # rows 64:128 (vhat is [ones | V])
                    bcr = rp.tile([128, 512], f32, tag="bcr", name="bcr")
                    nc.vector.reciprocal_approx_fast(bcr[0:HD, :],
                                                     po[0:HD, :])
                    nc.vector.tensor_mul(
                        oTcs[qc][64 * par:64 * par + 64, p, :],
                        po[HD:128, :], bcr[0:HD, :])

            def emit_c_chunk(n):
                oTc = oTcs[n]
                for m in range(nmt):
                    ps = psO.tile([128, 512], f32, tag="o", name="cps")
                    for j in range(NPAIR):
                        nc.tensor.matmul(
                            ps[:, :], wpt[:, j, m, :], oTc[:, j, :],
                            start=(j == 0), stop=(j == NPAIR - 1))
                    sb = ev.tile([128, 512], f32, tag="sb", name="sb")
                    nc.vector.tensor_copy(sb[:, :], ps[:, :])
                    nc.sync.dma_start(
                        out=yt[m * 128:(m + 1) * 128, n * 512:(n + 1) * 512],
                        in_=sb)

            def retire(st):
                while emit_pv_step(st):
                    pass
                emit_norm(st)
                if 'C' in stages and st["p"] == NPAIR - 1:
                    emit_c_chunk(st["qc"])

            # ---- main schedule ----
            units_on = 'B' in stages
            push_chunk(0)
            pump(10 ** 9)              # chunk-0 projections are the prologue
            if not apump:
                for n in range(1, nqc):
                    push_chunk(n)
                pump(10 ** 9)
            prev = None
            for n in range(nqc):
                if not units_on:
                    if apump and n + 1 < nqc:
                        push_chunk(n + 1)
                        pump(10 ** 9)
                    continue
                oTcs[n] = oTcp.tile([128, NPAIR, 512], bf16, tag="oTc",
                                    name=f"oTc{n}")
                if apump and n + 1 < nqc:
                    push_chunk(n + 1)
                steps_left = 4 * NPAIR * (n + 1)
                for p in range(NPAIR):
                    cur = new_state(p, n)
                    while True:
                        if prev is not None:
                            emit_pv_step(prev)
                        if not emit_s_step(cur):
                            break
                        rate = math.ceil(len(aq["items"]) / max(steps_left, 1))
                        pump(rate)
                        steps_left -= 1
                    if prev is not None:
                        retire(prev)
                    prev = cur
            if prev is not None:
                retire(prev)
            pump(10 ** 9)
    nc.compile()
    return nc


def _make_masks():
    import ml_dtypes
    k = np.arange(128)[:, None]
    q = np.arange(128)[None, :]
    return (q >= k).astype(ml_dtypes.bfloat16)


_NC_CACHE = {}


def _get_nc(tpc=T):
    if tpc not in _NC_CACHE:
        _NC_CACHE[tpc] = build_nc(tpc)
    return _NC_CACHE[tpc]


def make_in_maps(x, w_attn, w_proj):
    import ml_dtypes
    bf = ml_dtypes.bfloat16
    masks = _make_masks()
    nkt = T // 128
    ones = np.ones((128, nkt * HPC * HD), dtype=bf)
    in_maps = []
    for core in range(N_CORES):
        b, hh = core // 2, core % 2
        s = slice(hh * FS, (hh + 1) * FS)
        in_maps.append({
            "xt": np.ascontiguousarray(np.asarray(x[b]).T).astype(bf),
            "wq": np.ascontiguousarray(w_attn[:, s]).astype(bf),
            "wk": np.ascontiguousarray(w_attn[:, C:][:, s]).astype(bf),
            "wv": np.ascontiguousarray(w_attn[:, 2 * C:][:, s]).astype(bf),
            "wp": np.ascontiguousarray(w_proj[hh * FS:(hh + 1) * FS, :]).astype(bf),
            "mk": masks,
            "on": ones,
        })
    return in_maps


def kernel(x, w_attn, w_proj):
    nc = _get_nc(T)
    in_maps = make_in_maps(x, w_attn, w_proj)
    res = run_bass_kernel_spmd(nc, in_maps, list(range(N_CORES)))
    y = np.empty((B, T, C), np.float32)
    for b in range(B):
        yt = res.results[2 * b]["yt"] + res.results[2 * b + 1]["yt"]
        y[b] = yt.T
    return y
